# revision 48
# baseline (speedup 1.0000x reference)
"""Trainium2 Bass kernel for the non-local attention block (nn_Attention_79809082295188).

Reference computation (per batch b of 4, C=512 channels, N=4096 positions):
    theta = W_theta @ x          [64, N]
    phi   = W_phi @ x            [64, N]
    g     = W_g @ x              [256, N]
    scores[n, m] = theta[:, n] . phi[:, m]
    beta = softmax(scores, axis=m)
    o_mid[c, n] = sum_m g[c, m] beta[n, m]
    out = gamma * (W_o @ o_mid) + x

Sharding: 8 shards = batch(4) x query-half(2). Each core receives its batch's
full x with its own query half permuted to the FIRST 2048 columns (key order is
irrelevant to softmax attention), computes attention for those 2048 queries
against all 4096 keys, and writes a [512, 2048] output chunk.

On-core dataflow (matmuls bf16/f32r on PE, accumulation fp32 in PSUM):
  - scores are computed TRANSPOSED ([keys m on partitions, queries n free])
    so that exp(scores_T) tiles can be used directly as matmul lhsT for the
    attention*V contraction over m -- no big transposes anywhere.
  - the QK^T contraction is only 64 deep, so two key-chunks are packed onto
    the 128-row PE array concurrently via tile_position row groups. That
    needs theta duplicated on partitions 0:64 and 64:128 (theta2) and phi
    with even key-chunks on partitions 0:64 / odd on 64:128 (phi2); phi2 is
    produced directly by a col-group-packed pair of projection matmuls.
  - softmax denominator comes for free: a constant-1 column appended to g^T
    makes column 256 of the PV matmul output equal sum_m exp(scores_T[m, n]).
  - max-subtraction is skipped: scores are in [-12, 12], exp() is safe in fp32.
  - normalization is a per-partition scalar multiply, then a PE transpose of
    the [queries, 256] result back to [channels, queries] for the output proj.
"""

import sys

sys.path.insert(0, "/opt/trn_rl_repo")

from contextlib import ExitStack

import numpy as np
import ml_dtypes

import concourse.bass as bass
import concourse.bacc as bacc
import concourse.tile as tile
from concourse import mybir
from concourse.bass_utils import run_bass_kernel_spmd
from concourse.masks import make_identity

F32 = mybir.dt.float32
F32R = mybir.dt.float32r
BF16 = mybir.dt.bfloat16
F8 = mybir.dt.float8e4

# exp() is emitted as exp(s)*2^-EXP_SHIFT so it fits fp8e4 range (max ~240
# vs exp(score_max~11) ~ 60000); the scale cancels in the softmax ratio.
EXP_SHIFT = 9
EXP_BIAS = -float(EXP_SHIFT) * 0.6931471805599453
GT_STRIDE = 272  # g^T row stride in fp8 bytes: 257 columns padded to %16==0

C = 512          # channels
N = 4096         # sequence positions (keys per core)
P = 128          # partitions
CB = C // P      # 4 channel blocks
KD = 64          # theta/phi dim (C/8)
VD = 256         # g dim (C/2)
NQ = 2048        # queries per core
QB = 512         # query block
NQB = NQ // QB   # 4 query blocks
MT = N // P      # 32 key tiles
NCOL = 4         # x column tiles (for DMA/compute overlap)
COLW = N // NCOL # 1024
N_WARMUP = 16    # PE warmup matmuls to ride out the input DMA + HAM cold clock


def build_nc(gamma: float) -> bass.Bass:
    nc = bacc.Bacc(
        "TRN2",
        target_bir_lowering=False,
        debug=False,
        enable_asserts=False,
        num_devices=8,
    )
    x_in = nc.declare_dram_parameter("x", [C, N], BF16, isOutput=False)
    xq_in = nc.declare_dram_parameter("xq", [C, NQ], F32, isOutput=False)
    wqk_in = nc.declare_dram_parameter("wqk", [C, P], BF16, isOutput=False)
    # wph: [W_phi^T | 0] in cols 0:128, [0 | W_phi^T] in cols 128:256 -- lets
    # the even/odd key-chunk projections land on partitions 0:64 / 64:128 of
    # one PSUM tile via accumulation (walrus rejects col-tiled dst base 64).
    wph_in = nc.declare_dram_parameter("wph", [C, 2 * P], BF16, isOutput=False)
    wg_in = nc.declare_dram_parameter("wg", [C, VD], BF16, isOutput=False)
    wo_in = nc.declare_dram_parameter("wo", [VD, C], F8, isOutput=False)
    out_ext = nc.declare_dram_parameter("out", [C, NQ], F32, isOutput=True)

    x_r = x_in.rearrange("(cb p) (j w) -> p cb j w", p=P, w=COLW)
    xq_r = xq_in.rearrange("(cb p) n -> p cb n", p=P)
    out_r = out_ext.rearrange("(cb p) n -> p cb n", p=P)

    with tile.TileContext(nc) as tc, ExitStack() as ctx:
        const = ctx.enter_context(tc.tile_pool(name="const", bufs=1))
        big = ctx.enter_context(tc.tile_pool(name="big", bufs=1))
        eb = ctx.enter_context(tc.tile_pool(name="eb", bufs=2))
        wk = ctx.enter_context(tc.tile_pool(name="wk", bufs=2))
        recp = ctx.enter_context(tc.tile_pool(name="recp", bufs=4))
        outp = ctx.enter_context(tc.tile_pool(name="outp", bufs=4))
        # PSUM budget (8 banks): scores pairs 2x2 + small 2 + oproj 2
        psS = ctx.enter_context(tc.tile_pool(name="psS", bufs=2, space="PSUM"))
        psP = ctx.enter_context(tc.tile_pool(name="psP", bufs=2, space="PSUM"))
        psQ = ctx.enter_context(tc.tile_pool(name="psQ", bufs=2, space="PSUM"))

        # ---- PE warmup: keep TensorE busy during input DMA so HAM unthrottles
        dummy = const.tile([P, QB], BF16, tag="dummy")
        nc.gpsimd.memset(dummy, 0.0)
        # load the exp table-set during the DMA window, not at first real exp
        warm_exp = const.tile([P, 1], F32, tag="warm_exp")
        nc.scalar.activation(
            out=warm_exp,
            in_=dummy[:, 0:1],
            func=mybir.ActivationFunctionType.Exp,
        )
        for i in range(N_WARMUP):
            psw = psS.tile([P, 2 * QB], F32, tag="scores")
            nc.tensor.matmul(
                psw[:, 0:QB], lhsT=dummy[:, 0:P], rhs=dummy, start=True, stop=True
            )

        # ---- inputs: interleave x column tiles with the weights so the
        # first projection work unblocks as early as possible (wo last) ----
        xf = [
            big.tile([P, CB, COLW], BF16, tag=f"xf{j}", name=f"xf{j}")
            for j in range(NCOL)
        ]
        xq = big.tile([P, CB, NQ], F32, tag="xq")
        wqk_sb = const.tile([P, CB, P], BF16, tag="wqk")
        wph_sb = const.tile([P, CB, 2 * P], BF16, tag="wph")
        wg_sb = const.tile([P, CB, VD], BF16, tag="wg")
        wo_sb = const.tile([P, 2, C], F8, tag="wo")

        nc.sync.dma_start(out=xf[0], in_=x_r[:, :, 0, :])
        nc.sync.dma_start(out=wqk_sb, in_=wqk_in.rearrange("(cb p) k -> p cb k", p=P))
        nc.sync.dma_start(out=xf[1], in_=x_r[:, :, 1, :])
        nc.sync.dma_start(out=wph_sb, in_=wph_in.rearrange("(cb p) k -> p cb k", p=P))
        nc.sync.dma_start(out=wg_sb, in_=wg_in.rearrange("(cb p) k -> p cb k", p=P))
        nc.sync.dma_start(out=xf[2], in_=x_r[:, :, 2, :])
        nc.sync.dma_start(out=xf[3], in_=x_r[:, :, 3, :])
        nc.sync.dma_start(out=wo_sb, in_=wo_in.rearrange("(cb p) k -> p cb k", p=P))
        nc.sync.dma_start(out=xq, in_=xq_r)
        ident = const.tile([P, P], BF16, tag="ident")
        make_identity(nc, ident)
        exp_bias = const.tile([P, 1], F32, tag="exp_bias")
        nc.vector.memset(exp_bias, EXP_BIAS)

        def xcols(lo, hi):
            """AP for x columns [lo, hi) -- must lie within one column tile."""
            j = lo // COLW
            assert hi <= (j + 1) * COLW
            return xf[j][:, :, lo - j * COLW : hi - j * COLW]

        # theta duplicated on both partition halves (for row-packed QK^T)
        theta2 = big.tile([P, NQ], BF16, tag="theta2")
        # phi2: even key-chunks on partitions 0:64, odd on 64:128;
        # free col block j holds key chunks (2j, 2j+1)
        phi2 = big.tile([P, N // 2], BF16, tag="phi2")
        gt = big.tile([P, MT, GT_STRIDE], F8, tag="gt")

        def theta_proj(q4):
            """theta for query cols q4*512.. (wqk = [W_theta^T | W_theta^T])."""
            ps = psQ.tile([P, QB], F32, tag="oproj")
            for cb in range(CB):
                nc.tensor.matmul(
                    ps,
                    lhsT=wqk_sb[:, cb, :],
                    rhs=xcols(q4 * QB, (q4 + 1) * QB)[:, cb, :],
                    start=(cb == 0),
                    stop=(cb == CB - 1),
                )
            nc.vector.tensor_copy(theta2[:, q4 * QB : (q4 + 1) * QB], ps)

        def phi_proj(t):
            """phi2 cols [t*512,(t+1)*512) = key chunks 8t..8t+7: even chunks
            to partitions 0:64, odd to 64:128, via zero-padded lhsT halves
            accumulating into one PSUM tile."""
            ps = psQ.tile([P, QB], F32, tag="oproj")
            xt3 = xf[t].rearrange("p cb (pr two w) -> p cb pr two w", two=2, w=P)
            for cb in range(CB):
                nc.tensor.matmul(
                    ps,
                    lhsT=wph_sb[:, cb, 0:P],
                    rhs=xt3[:, cb, :, 0, :],
                    start=(cb == 0),
                    stop=False,
                )
            for cb in range(CB):
                nc.tensor.matmul(
                    ps,
                    lhsT=wph_sb[:, cb, P : 2 * P],
                    rhs=xt3[:, cb, :, 1, :],
                    start=False,
                    stop=(cb == CB - 1),
                )
            nc.vector.tensor_copy(phi2[:, t * QB : (t + 1) * QB], ps)

        def gt_proj(mi):
            """gt[m, c] = sum_cin x[cin, m] * wg[cin, c], stored fp8."""
            ps = psP.tile([P, VD], F32, tag="small")
            for cb in range(CB):
                nc.tensor.matmul(
                    ps,
                    lhsT=xcols(mi * P, (mi + 1) * P)[:, cb, :],
                    rhs=wg_sb[:, cb, :],
                    start=(cb == 0),
                    stop=(cb == CB - 1),
                )
            nc.vector.tensor_copy(gt[:, mi, 0:VD], ps)

        # ---- scores + exp: pairs of key-chunks -> one 1024-wide exp ----
        def scores_pair(b, et, j):
            """exp(scores^T)*2^-EXP_SHIFT (fp8) for query block b, key chunks
            2j, 2j+1 (one row-group-packed matmul pair, one exp)."""
            ps = psS.tile([P, 2 * QB], F32, tag="scores", name=f"sc{b}_{j}")
            nc.tensor.matmul(
                ps[:, 0:QB],
                lhsT=phi2[0:KD, j * P : (j + 1) * P],
                rhs=theta2[0:KD, b * QB : (b + 1) * QB],
                start=True,
                stop=True,
                tile_position=(0, 0),
            )
            nc.tensor.matmul(
                ps[:, QB : 2 * QB],
                lhsT=phi2[KD:P, j * P : (j + 1) * P],
                rhs=theta2[KD:P, b * QB : (b + 1) * QB],
                start=True,
                stop=True,
                tile_position=(KD, 0),
            )
            nc.scalar.activation(
                out=et[:, 2 * j : 2 * j + 2, :],
                in_=ps.rearrange("p (k w) -> p k w", k=2),
                func=mybir.ActivationFunctionType.Exp,
                bias=exp_bias,
            )

        def new_et(b):
            return eb.tile([P, MT, QB], F8, tag="expT", name=f"et{b}")

        # emit per x-column-tile so compute unblocks as each DMA lands;
        # block 0's scores/exp quads are folded in as their phi cols appear
        et0 = new_et(0)
        for t in range(NCOL):
            if t < 2:
                theta_proj(2 * t)
                theta_proj(2 * t + 1)
            phi_proj(t)
            for j in range(4 * t, 4 * t + 4):
                scores_pair(0, et0, j)
            for mi in range(8 * t, 8 * t + 8):
                gt_proj(mi)
            if t == 0:
                # gt ones column; also needed before any PV
                nc.vector.memset(gt[:, :, VD : VD + 1], 1.0)

        def pv_block(b, et, et_next):
            omidT = wk.tile([P, NQB, VD], BF16, tag="omidT")
            omid = wk.tile([P, 2, QB], F8, tag="omid")

            def transpose_qc(qc):
                # [queries, 256] -> [256, queries]
                for oc2 in range(2):
                    pst = psQ.tile([P, P], BF16, tag="oproj")
                    nc.tensor.transpose(
                        pst, omidT[:, qc, oc2 * P : (oc2 + 1) * P], ident
                    )
                    nc.vector.tensor_copy(omid[:, oc2, qc * P : (qc + 1) * P], pst)

            for qc in range(NQB):
                # next block's score pairs, interleaved 1:4 with the PV
                # matmuls so the scalar engine's exp stream never starves
                pso = psP.tile([P, VD + 1], F32, tag="small")
                for j2 in range(MT // 2):
                    if j2 % 4 == 0 and et_next is not None:
                        scores_pair(b + 1, et_next, 4 * qc + j2 // 4)
                    nc.tensor.matmul(
                        pso,
                        lhsT=et[:, 2 * j2 : 2 * j2 + 2, qc * P : (qc + 1) * P],
                        rhs=gt[:, 2 * j2 : 2 * j2 + 2, 0 : VD + 1],
                        start=(j2 == 0),
                        stop=(j2 == MT // 2 - 1),
                        perf_mode=mybir.MatmulPerfMode.DoubleRow,
                    )
                rec = recp.tile([P, 1], F32, tag="rec")
                nc.vector.reciprocal(rec, pso[:, VD : VD + 1])
                nc.vector.tensor_scalar(
                    omidT[:, qc, :],
                    pso[:, 0:VD],
                    scalar1=rec,
                    scalar2=8.0,
                    op0=mybir.AluOpType.mult,
                    op1=mybir.AluOpType.mult,
                )
                if qc > 0:
                    transpose_qc(qc - 1)  # deps long met -> no PE stall
            transpose_qc(NQB - 1)
            # output projection + residual: fp8 DoubleRow, one pass per oc;
            # the host's wo x16 and the normalize's x8 cancel via gamma/128
            for oc in range(CB):
                psq = psQ.tile([P, QB], F32, tag="oproj")
                nc.tensor.matmul(
                    psq,
                    lhsT=wo_sb[:, :, oc * P : (oc + 1) * P],
                    rhs=omid,
                    start=True,
                    stop=True,
                    perf_mode=mybir.MatmulPerfMode.DoubleRow,
                )
                ot = outp.tile([P, QB], F32, tag="out")
                nc.vector.scalar_tensor_tensor(
                    out=ot,
                    in0=psq,
                    scalar=gamma / 128.0,
                    in1=xq[:, oc, b * QB : (b + 1) * QB],
                    op0=mybir.AluOpType.mult,
                    op1=mybir.AluOpType.add,
                )
                nc.sync.dma_start(out=out_r[:, oc, b * QB : (b + 1) * QB], in_=ot)

        et = et0
        for b in range(NQB):
            et_next = new_et(b + 1) if b + 1 < NQB else None
            pv_block(b, et, et_next)
            et = et_next

    nc.compile()
    return nc


_CACHE: dict = {}


def _get_nc(gamma: float) -> bass.Bass:
    if gamma not in _CACHE:
        _CACHE[gamma] = build_nc(gamma)
    return _CACHE[gamma]


def _prep_in_maps(x, W_theta, W_phi, W_g, W_o):
    x = np.ascontiguousarray(np.asarray(x, dtype=np.float32))
    bf16 = ml_dtypes.bfloat16
    wth = np.asarray(W_theta, np.float32).T
    wqk = np.ascontiguousarray(np.concatenate([wth, wth], axis=1)).astype(bf16)
    wphT = np.asarray(W_phi, np.float32).T
    wph = np.zeros((C, 2 * P), np.float32)
    wph[:, 0:KD] = wphT
    wph[:, P + KD : 2 * P] = wphT
    wph = wph.astype(bf16)
    wg = np.ascontiguousarray(np.asarray(W_g, np.float32).T).astype(bf16)
    wo = np.ascontiguousarray(np.asarray(W_o, np.float32).T * 16.0).astype(
        ml_dtypes.float8_e4m3
    )
    in_maps = []
    for core in range(8):
        b, h = divmod(core, 2)
        xb = x[b]
        x_perm = np.ascontiguousarray(
            np.concatenate(
                [xb[:, h * NQ : (h + 1) * NQ], xb[:, (1 - h) * NQ : (2 - h) * NQ]],
                axis=1,
            )
        )
        xq = np.ascontiguousarray(x_perm[:, 0:NQ])
        in_maps.append(
            {
                "x": x_perm.astype(bf16),
                "xq": xq,
                "wqk": wqk,
                "wph": wph,
                "wg": wg,
                "wo": wo,
            }
        )
    return in_maps


def _run(x, W_theta, W_phi, W_g, W_o, gamma, trace=False):
    nc = _get_nc(float(gamma))
    in_maps = _prep_in_maps(x, W_theta, W_phi, W_g, W_o)
    # the first execution of a fresh NEFF occasionally hits a transient
    # NRT_EXEC_UNIT_UNRECOVERABLE on this fabric; a retry recovers it
    last_err = None
    for attempt in range(3):
        try:
            res = run_bass_kernel_spmd(nc, in_maps, list(range(8)), trace=trace)
            break
        except Exception as e:  # noqa: BLE001 - device-side flake, retry
            last_err = e
            import time

            time.sleep(2.0)
    else:
        raise last_err
    out = np.empty((4, C, N), np.float32)
    for core in range(8):
        b, h = divmod(core, 2)
        out[b][:, h * NQ : (h + 1) * NQ] = res.results[core]["out"]
    return out, res


def kernel(x, W_theta, W_phi, W_g, W_o, gamma):
    out, _ = _run(x, W_theta, W_phi, W_g, W_o, gamma)
    return out


# revision 49
# speedup vs baseline: 1.1510x; 1.1510x over previous
"""Trainium2 Bass kernel for the non-local attention block (nn_Attention_79809082295188).

Reference computation (per batch b of 4, C=512 channels, N=4096 positions):
    theta = W_theta @ x          [64, N]
    phi   = W_phi @ x            [64, N]
    g     = W_g @ x              [256, N]
    scores[n, m] = theta[:, n] . phi[:, m]
    beta = softmax(scores, axis=m)
    o_mid[c, n] = sum_m g[c, m] beta[n, m]
    out = gamma * (W_o @ o_mid) + x

Sharding: 8 shards = batch(4) x query-half(2). Each core receives its batch's
full x with its own query half permuted to the FIRST 2048 columns (key order is
irrelevant to softmax attention), computes attention for those 2048 queries
against all 4096 keys, and writes a [512, 2048] output chunk.

On-core dataflow (matmuls bf16/f32r on PE, accumulation fp32 in PSUM):
  - scores are computed TRANSPOSED ([keys m on partitions, queries n free])
    so that exp(scores_T) tiles can be used directly as matmul lhsT for the
    attention*V contraction over m -- no big transposes anywhere.
  - the QK^T contraction is only 64 deep, so two key-chunks are packed onto
    the 128-row PE array concurrently via tile_position row groups. That
    needs theta duplicated on partitions 0:64 and 64:128 (theta2) and phi
    with even key-chunks on partitions 0:64 / odd on 64:128 (phi2); phi2 is
    produced directly by a col-group-packed pair of projection matmuls.
  - softmax denominator comes for free: a constant-1 column appended to g^T
    makes column 256 of the PV matmul output equal sum_m exp(scores_T[m, n]).
  - max-subtraction is skipped: scores are in [-12, 12], exp() is safe in fp32.
  - normalization is a per-partition scalar multiply, then a PE transpose of
    the [queries, 256] result back to [channels, queries] for the output proj.
"""

import sys

sys.path.insert(0, "/opt/trn_rl_repo")

from contextlib import ExitStack

import numpy as np
import ml_dtypes

import concourse.bass as bass
import concourse.bacc as bacc
import concourse.tile as tile
from concourse import mybir
from concourse.bass_utils import run_bass_kernel_spmd
from concourse.masks import make_identity

F32 = mybir.dt.float32
F32R = mybir.dt.float32r
BF16 = mybir.dt.bfloat16
F8 = mybir.dt.float8e4

# exp() is emitted as exp(s)*2^-EXP_SHIFT so it fits fp8e4 range (max ~240
# vs exp(score_max~11) ~ 60000); the scale cancels in the softmax ratio.
EXP_SHIFT = 9
EXP_BIAS = -float(EXP_SHIFT) * 0.6931471805599453
GT_STRIDE = 272  # g^T row stride in fp8 bytes: 257 columns padded to %16==0

C = 512          # channels
N = 4096         # sequence positions (keys per core)
P = 128          # partitions
CB = C // P      # 4 channel blocks
KD = 64          # theta/phi dim (C/8)
VD = 256         # g dim (C/2)
NQ = 2048        # queries per core
QB = 512         # query block
NQB = NQ // QB   # 4 query blocks
MT = N // P      # 32 key tiles
NCOL = 4         # x column tiles (for DMA/compute overlap)
COLW = N // NCOL # 1024
N_WARMUP = 16    # PE warmup matmuls to ride out the input DMA + HAM cold clock


def build_nc(gamma: float) -> bass.Bass:
    nc = bacc.Bacc(
        "TRN2",
        target_bir_lowering=False,
        debug=False,
        enable_asserts=False,
        num_devices=8,
    )
    x_in = nc.declare_dram_parameter("x", [C, N], BF16, isOutput=False)
    xq_in = nc.declare_dram_parameter("xq", [C, NQ], F32, isOutput=False)
    wqk_in = nc.declare_dram_parameter("wqk", [C, P], BF16, isOutput=False)
    # wph: [W_phi^T | 0] in cols 0:128, [0 | W_phi^T] in cols 128:256 -- lets
    # the even/odd key-chunk projections land on partitions 0:64 / 64:128 of
    # one PSUM tile via accumulation (walrus rejects col-tiled dst base 64).
    wph_in = nc.declare_dram_parameter("wph", [C, 2 * P], BF16, isOutput=False)
    wg_in = nc.declare_dram_parameter("wg", [C, VD], BF16, isOutput=False)
    wo_in = nc.declare_dram_parameter("wo", [VD, C], BF16, isOutput=False)
    out_ext = nc.declare_dram_parameter("out", [C, NQ], F32, isOutput=True)

    x_r = x_in.rearrange("(cb p) (j w) -> p cb j w", p=P, w=COLW)
    xq_r = xq_in.rearrange("(cb p) n -> p cb n", p=P)
    out_r = out_ext.rearrange("(cb p) n -> p cb n", p=P)

    with tile.TileContext(nc) as tc, ExitStack() as ctx:
        const = ctx.enter_context(tc.tile_pool(name="const", bufs=1))
        big = ctx.enter_context(tc.tile_pool(name="big", bufs=1))
        eb = ctx.enter_context(tc.tile_pool(name="eb", bufs=2))
        wk = ctx.enter_context(tc.tile_pool(name="wk", bufs=2))
        recp = ctx.enter_context(tc.tile_pool(name="recp", bufs=4))
        outp = ctx.enter_context(tc.tile_pool(name="outp", bufs=4))
        # PSUM budget (8 banks): scores pairs 2x2 + small 2 + oproj 2
        psS = ctx.enter_context(tc.tile_pool(name="psS", bufs=2, space="PSUM"))
        psP = ctx.enter_context(tc.tile_pool(name="psP", bufs=2, space="PSUM"))
        psQ = ctx.enter_context(tc.tile_pool(name="psQ", bufs=2, space="PSUM"))

        # ---- PE warmup: keep TensorE busy during input DMA so HAM unthrottles
        dummy = const.tile([P, QB], BF16, tag="dummy")
        nc.gpsimd.memset(dummy, 0.0)
        # load the exp table-set during the DMA window, not at first real exp
        warm_exp = const.tile([P, 1], F32, tag="warm_exp")
        nc.scalar.activation(
            out=warm_exp,
            in_=dummy[:, 0:1],
            func=mybir.ActivationFunctionType.Exp,
        )
        for i in range(N_WARMUP):
            psw = psS.tile([P, 2 * QB], F32, tag="scores")
            nc.tensor.matmul(
                psw[:, 0:QB], lhsT=dummy[:, 0:P], rhs=dummy, start=True, stop=True
            )

        # ---- inputs: interleave x column tiles with the weights so the
        # first projection work unblocks as early as possible (wo last) ----
        xf = [
            big.tile([P, CB, COLW], BF16, tag=f"xf{j}", name=f"xf{j}")
            for j in range(NCOL)
        ]
        xq = big.tile([P, CB, NQ], F32, tag="xq")
        wqk_sb = const.tile([P, CB, P], BF16, tag="wqk")
        wph_sb = const.tile([P, CB, 2 * P], BF16, tag="wph")
        wg_sb = const.tile([P, CB, VD], BF16, tag="wg")
        wo_sb = const.tile([P, 2, C], BF16, tag="wo")

        nc.sync.dma_start(out=xf[0], in_=x_r[:, :, 0, :])
        nc.sync.dma_start(out=wqk_sb, in_=wqk_in.rearrange("(cb p) k -> p cb k", p=P))
        nc.sync.dma_start(out=xf[1], in_=x_r[:, :, 1, :])
        nc.sync.dma_start(out=wph_sb, in_=wph_in.rearrange("(cb p) k -> p cb k", p=P))
        nc.sync.dma_start(out=wg_sb, in_=wg_in.rearrange("(cb p) k -> p cb k", p=P))
        nc.sync.dma_start(out=xf[2], in_=x_r[:, :, 2, :])
        nc.sync.dma_start(out=xf[3], in_=x_r[:, :, 3, :])
        nc.sync.dma_start(out=wo_sb, in_=wo_in.rearrange("(cb p) k -> p cb k", p=P))
        nc.sync.dma_start(out=xq, in_=xq_r)
        ident = const.tile([P, P], BF16, tag="ident")
        make_identity(nc, ident)
        exp_bias = const.tile([P, 1], F32, tag="exp_bias")
        nc.vector.memset(exp_bias, EXP_BIAS)

        def xcols(lo, hi):
            """AP for x columns [lo, hi) -- must lie within one column tile."""
            j = lo // COLW
            assert hi <= (j + 1) * COLW
            return xf[j][:, :, lo - j * COLW : hi - j * COLW]

        # theta duplicated on both partition halves (for row-packed QK^T)
        theta2 = big.tile([P, NQ], BF16, tag="theta2")
        # phi2: even key-chunks on partitions 0:64, odd on 64:128;
        # free col block j holds key chunks (2j, 2j+1)
        phi2 = big.tile([P, N // 2], BF16, tag="phi2")
        gt = big.tile([P, MT, GT_STRIDE], F8, tag="gt")

        def theta_proj(q4):
            """theta for query cols q4*512.. (wqk = [W_theta^T | W_theta^T])."""
            ps = psQ.tile([P, QB], F32, tag="oproj")
            for cb in range(CB):
                nc.tensor.matmul(
                    ps,
                    lhsT=wqk_sb[:, cb, :],
                    rhs=xcols(q4 * QB, (q4 + 1) * QB)[:, cb, :],
                    start=(cb == 0),
                    stop=(cb == CB - 1),
                )
            nc.vector.tensor_copy(theta2[:, q4 * QB : (q4 + 1) * QB], ps)

        def phi_proj(t):
            """phi2 cols [t*512,(t+1)*512) = key chunks 8t..8t+7: even chunks
            to partitions 0:64, odd to 64:128, via zero-padded lhsT halves
            accumulating into one PSUM tile."""
            ps = psQ.tile([P, QB], F32, tag="oproj")
            xt3 = xf[t].rearrange("p cb (pr two w) -> p cb pr two w", two=2, w=P)
            for cb in range(CB):
                nc.tensor.matmul(
                    ps,
                    lhsT=wph_sb[:, cb, 0:P],
                    rhs=xt3[:, cb, :, 0, :],
                    start=(cb == 0),
                    stop=False,
                )
            for cb in range(CB):
                nc.tensor.matmul(
                    ps,
                    lhsT=wph_sb[:, cb, P : 2 * P],
                    rhs=xt3[:, cb, :, 1, :],
                    start=False,
                    stop=(cb == CB - 1),
                )
            nc.vector.tensor_copy(phi2[:, t * QB : (t + 1) * QB], ps)

        def gt_proj(mi):
            """gt[m, c] = sum_cin x[cin, m] * wg[cin, c], stored fp8."""
            ps = psP.tile([P, VD], F32, tag="small")
            for cb in range(CB):
                nc.tensor.matmul(
                    ps,
                    lhsT=xcols(mi * P, (mi + 1) * P)[:, cb, :],
                    rhs=wg_sb[:, cb, :],
                    start=(cb == 0),
                    stop=(cb == CB - 1),
                )
            nc.vector.tensor_copy(gt[:, mi, 0:VD], ps)

        # ---- scores + exp: pairs of key-chunks -> one 1024-wide exp ----
        def scores_pair(b, et, j):
            """exp(scores^T)*2^-EXP_SHIFT (fp8) for query block b, key chunks
            2j, 2j+1 (one row-group-packed matmul pair, one exp)."""
            ps = psS.tile([P, 2 * QB], F32, tag="scores", name=f"sc{b}_{j}")
            nc.tensor.matmul(
                ps[:, 0:QB],
                lhsT=phi2[0:KD, j * P : (j + 1) * P],
                rhs=theta2[0:KD, b * QB : (b + 1) * QB],
                start=True,
                stop=True,
                tile_position=(0, 0),
            )
            nc.tensor.matmul(
                ps[:, QB : 2 * QB],
                lhsT=phi2[KD:P, j * P : (j + 1) * P],
                rhs=theta2[KD:P, b * QB : (b + 1) * QB],
                start=True,
                stop=True,
                tile_position=(KD, 0),
            )
            nc.scalar.activation(
                out=et[:, 2 * j : 2 * j + 2, :],
                in_=ps.rearrange("p (k w) -> p k w", k=2),
                func=mybir.ActivationFunctionType.Exp,
                bias=exp_bias,
            )

        def new_et(b):
            return eb.tile([P, MT, QB], F8, tag="expT", name=f"et{b}")

        # emit per x-column-tile so compute unblocks as each DMA lands;
        # block 0's scores/exp quads are folded in as their phi cols appear
        et0 = new_et(0)
        for t in range(NCOL):
            if t < 2:
                theta_proj(2 * t)
                theta_proj(2 * t + 1)
            phi_proj(t)
            for j in range(4 * t, 4 * t + 4):
                scores_pair(0, et0, j)
            for mi in range(8 * t, 8 * t + 8):
                gt_proj(mi)
            if t == 0:
                # gt ones column; also needed before any PV
                nc.vector.memset(gt[:, :, VD : VD + 1], 1.0)

        def pv_block(b, et, et_next):
            omidT = wk.tile([P, NQB, VD], BF16, tag="omidT")
            omid = wk.tile([P, 2, QB], BF16, tag="omid")

            def transpose_qc(qc):
                # [queries, 256] -> [256, queries]
                for oc2 in range(2):
                    pst = psQ.tile([P, P], BF16, tag="oproj")
                    nc.tensor.transpose(
                        pst, omidT[:, qc, oc2 * P : (oc2 + 1) * P], ident
                    )
                    nc.vector.tensor_copy(omid[:, oc2, qc * P : (qc + 1) * P], pst)

            for qc in range(NQB):
                # next block's score pairs, interleaved 1:4 with the PV
                # matmuls so the scalar engine's exp stream never starves
                pso = psP.tile([P, VD + 1], F32, tag="small")
                for j2 in range(MT // 2):
                    if j2 % 4 == 0 and et_next is not None:
                        scores_pair(b + 1, et_next, 4 * qc + j2 // 4)
                    nc.tensor.matmul(
                        pso,
                        lhsT=et[:, 2 * j2 : 2 * j2 + 2, qc * P : (qc + 1) * P],
                        rhs=gt[:, 2 * j2 : 2 * j2 + 2, 0 : VD + 1],
                        start=(j2 == 0),
                        stop=(j2 == MT // 2 - 1),
                        perf_mode=mybir.MatmulPerfMode.DoubleRow,
                    )
                rec = recp.tile([P, 1], F32, tag="rec")
                nc.vector.reciprocal(rec, pso[:, VD : VD + 1])
                nc.vector.tensor_scalar_mul(omidT[:, qc, :], pso[:, 0:VD], rec)
                if qc > 0:
                    transpose_qc(qc - 1)  # deps long met -> no PE stall
            transpose_qc(NQB - 1)
            # output projection + residual
            for oc in range(CB):
                psq = psQ.tile([P, QB], F32, tag="oproj")
                for c2 in range(2):
                    nc.tensor.matmul(
                        psq,
                        lhsT=wo_sb[:, c2, oc * P : (oc + 1) * P],
                        rhs=omid[:, c2, :],
                        start=(c2 == 0),
                        stop=(c2 == 1),
                    )
                ot = outp.tile([P, QB], F32, tag="out")
                nc.vector.scalar_tensor_tensor(
                    out=ot,
                    in0=psq,
                    scalar=gamma,
                    in1=xq[:, oc, b * QB : (b + 1) * QB],
                    op0=mybir.AluOpType.mult,
                    op1=mybir.AluOpType.add,
                )
                nc.sync.dma_start(out=out_r[:, oc, b * QB : (b + 1) * QB], in_=ot)

        et = et0
        for b in range(NQB):
            et_next = new_et(b + 1) if b + 1 < NQB else None
            pv_block(b, et, et_next)
            et = et_next

    nc.compile()
    return nc


_CACHE: dict = {}


def _get_nc(gamma: float) -> bass.Bass:
    if gamma not in _CACHE:
        _CACHE[gamma] = build_nc(gamma)
    return _CACHE[gamma]


def _prep_in_maps(x, W_theta, W_phi, W_g, W_o):
    x = np.ascontiguousarray(np.asarray(x, dtype=np.float32))
    bf16 = ml_dtypes.bfloat16
    wth = np.asarray(W_theta, np.float32).T
    wqk = np.ascontiguousarray(np.concatenate([wth, wth], axis=1)).astype(bf16)
    wphT = np.asarray(W_phi, np.float32).T
    wph = np.zeros((C, 2 * P), np.float32)
    wph[:, 0:KD] = wphT
    wph[:, P + KD : 2 * P] = wphT
    wph = wph.astype(bf16)
    wg = np.ascontiguousarray(np.asarray(W_g, np.float32).T).astype(bf16)
    wo = np.ascontiguousarray(np.asarray(W_o, np.float32).T).astype(
        ml_dtypes.bfloat16
    )
    in_maps = []
    for core in range(8):
        b, h = divmod(core, 2)
        xb = x[b]
        x_perm = np.ascontiguousarray(
            np.concatenate(
                [xb[:, h * NQ : (h + 1) * NQ], xb[:, (1 - h) * NQ : (2 - h) * NQ]],
                axis=1,
            )
        )
        xq = np.ascontiguousarray(x_perm[:, 0:NQ])
        in_maps.append(
            {
                "x": x_perm.astype(bf16),
                "xq": xq,
                "wqk": wqk,
                "wph": wph,
                "wg": wg,
                "wo": wo,
            }
        )
    return in_maps


def _run(x, W_theta, W_phi, W_g, W_o, gamma, trace=False):
    nc = _get_nc(float(gamma))
    in_maps = _prep_in_maps(x, W_theta, W_phi, W_g, W_o)
    # the first execution of a fresh NEFF occasionally hits a transient
    # NRT_EXEC_UNIT_UNRECOVERABLE on this fabric; a retry recovers it
    last_err = None
    for attempt in range(3):
        try:
            res = run_bass_kernel_spmd(nc, in_maps, list(range(8)), trace=trace)
            break
        except Exception as e:  # noqa: BLE001 - device-side flake, retry
            last_err = e
            import time

            time.sleep(2.0)
    else:
        raise last_err
    out = np.empty((4, C, N), np.float32)
    for core in range(8):
        b, h = divmod(core, 2)
        out[b][:, h * NQ : (h + 1) * NQ] = res.results[core]["out"]
    return out, res


def kernel(x, W_theta, W_phi, W_g, W_o, gamma):
    out, _ = _run(x, W_theta, W_phi, W_g, W_o, gamma)
    return out
